# revision 15
# baseline (speedup 1.0000x reference)
"""CrossCovarianceAttn Trainium2 kernel.

Data-parallel over B=8 across 8 NeuronCores; each core runs the full model on
one batch element. All big matmuls run in fp32r (tf32-class, 1 cyc/row for
moving dim >= 256); PE transposes run in fp32 (exact). Norms over the token
dim come from Gram-matrix diagonals computed on the PE (avoids partition
reductions).
"""
import sys

sys.path.insert(0, "/opt/trn_rl_repo")

import numpy as np

import concourse.bass as bass
import concourse.mybir as mybir
import concourse.tile as tile
from concourse import bacc
from concourse.bass_utils import run_bass_kernel_spmd
from concourse.masks import make_identity

FP32 = mybir.dt.float32
FP32R = mybir.dt.float32r

N_TOK = 4096
C = 768
H = 8
HD = 96  # head dim
C3 = 3 * C  # 2304
TOK_TILE = 512
N_TILES = N_TOK // TOK_TILE  # 8
CHUNKS = TOK_TILE // 128  # 4
KK = C // 128  # 6
EPS = 1e-12

_CACHED_NC = None


def _qk_perm_strips(m):
    """For w_qkv c3-block m (rows 128m..128m+127, all in the q|k region),
    return strips (j_start, length, dst_col) mapping block-local row j to the
    head-interleaved column layout: q row (96h+d) -> 192h+d, k row -> 192h+96+d.
    """
    strips = []
    j = 0
    while j < 128:
        c3 = 128 * m + j
        if c3 < C:  # q region
            h, d = divmod(c3, HD)
            dst = 192 * h + d
            run = min(128 - j, HD - d)
        else:  # k region
            h, d = divmod(c3 - C, HD)
            dst = 192 * h + HD + d
            run = min(128 - j, HD - d)
        strips.append((j, run, dst))
        j += run
    return strips


def build_nc():
    nc = bacc.Bacc("TRN2", target_bir_lowering=False, debug=False, num_devices=8)

    x_d = nc.dram_tensor("x", (N_TOK, C), FP32, kind="ExternalInput").ap()
    wqkv_d = nc.dram_tensor("w_qkv", (C3, C), FP32, kind="ExternalInput").ap()
    temp_d = nc.dram_tensor("temperature", (H, 1, 1), FP32, kind="ExternalInput").ap()
    wproj_d = nc.dram_tensor("w_proj", (C, C), FP32, kind="ExternalInput").ap()
    bproj_d = nc.dram_tensor("b_proj", (C,), FP32, kind="ExternalInput").ap()
    out_d = nc.dram_tensor("out", (N_TOK, C), FP32, kind="ExternalOutput").ap()

    with tile.TileContext(nc) as tc:
        _build(tc, nc, x_d, wqkv_d, temp_d, wproj_d, bproj_d, out_d)
    nc.compile()
    return nc


def _build(tc, nc, x_d, wqkv_d, temp_d, wproj_d, bproj_d, out_d):
    import contextlib

    ctx = contextlib.ExitStack()
    with ctx:
        singles = ctx.enter_context(tc.tile_pool(name="singles", bufs=1))
        dram = ctx.enter_context(tc.tile_pool(name="dram", bufs=1, space="DRAM"))
        ps_tr = ctx.enter_context(tc.tile_pool(name="ps_tr", bufs=2, space="PSUM"))

        ident = singles.tile([128, 128], FP32)
        make_identity(nc, ident)

        # bias/temperature broadcast across partitions at load time (DMA
        # supports zero-step partition APs; DVE does not)
        b_all = singles.tile([128, C], FP32)
        nc.gpsimd.dma_start(
            b_all, bass.AP(tensor=bproj_d.tensor, offset=bproj_d.offset,
                           ap=[[0, 128], [1, C]]))
        temp_all = singles.tile([HD, H], FP32)
        nc.gpsimd.dma_start(
            temp_all, bass.AP(tensor=temp_d.tensor, offset=temp_d.offset,
                              ap=[[0, HD], [1, H]]))

        # persistent across phases
        cg_accum = singles.tile([HD, H, 512], FP32)
        nc.vector.memset(cg_accum, 0.0)
        zpad = singles.tile([128, 256], FP32)
        nc.vector.memset(zpad, 0.0)
        attnT = singles.tile([HD, H, HD], FP32R)

        v_dram = dram.tile([N_TOK, C], FP32)

        # ---------------- phase 0: weight prep ----------------
        with tc.tile_pool(name="wload", bufs=2) as wload, \
             tc.tile_pool(name="wqk_pool", bufs=1) as wqk_pool:
            w_qkT = wqk_pool.tile([128, KK, 2 * C], FP32R)
            w_vT = wqk_pool.tile([128, KK, C], FP32R)

            # w_qkv: 18 c3-blocks of 128 rows; transpose each (128,128) piece
            for m in range(C3 // 128):
                w_blk = wload.tile([128, C], FP32, name="w_blk")
                nc.sync.dma_start(w_blk, wqkv_d[m * 128:(m + 1) * 128, :])
                for kk in range(KK):
                    tps = ps_tr.tile([128, 128], FP32, name="tps", tag="tr")
                    nc.tensor.transpose(tps, w_blk[:, kk * 128:(kk + 1) * 128], ident)
                    if m < 12:  # q|k region -> head-interleaved columns
                        for (j0, run, dst) in _qk_perm_strips(m):
                            nc.vector.tensor_copy(
                                w_qkT[:, kk, dst:dst + run], tps[:, j0:j0 + run])
                    else:  # v region, natural order
                        base = m * 128 - 2 * C
                        nc.vector.tensor_copy(
                            w_vT[:, kk, base:base + 128], tps)

            # ---------------- phase 1: qkv + covariance/Gram ----------------
            with tc.tile_pool(name="xin", bufs=2) as xin, \
                 tc.tile_pool(name="xtp", bufs=2) as xtp, \
                 tc.tile_pool(name="qkp", bufs=1) as qkp, \
                 tc.tile_pool(name="vp", bufs=2) as vp, \
                 tc.tile_pool(name="ps_mm", bufs=3, space="PSUM") as ps_mm, \
                 tc.tile_pool(name="ps_cg", bufs=2, space="PSUM") as ps_cg:
                for t in range(N_TILES):
                    t0 = t * TOK_TILE
                    xT_t = xtp.tile([128, KK, TOK_TILE], FP32R, name="xT_t")
                    qk_t = qkp.tile([128, CHUNKS, 1600], FP32R, name="qk_t")
                    nc.vector.tensor_copy(
                        qk_t[:, :, 1536:],
                        zpad.rearrange("p (c w) -> p c w", c=CHUNKS))
                    v_t = vp.tile([128, CHUNKS, C], FP32, name="v_t")

                    for c in range(CHUNKS):
                        x_c = xin.tile([128, C], FP32, name="x_c")
                        nc.sync.dma_start(
                            x_c, x_d[t0 + c * 128: t0 + (c + 1) * 128, :])
                        for kk in range(KK):
                            xps = ps_tr.tile([128, 128], FP32, name="xps", tag="tr")
                            nc.tensor.transpose(
                                xps, x_c[:, kk * 128:(kk + 1) * 128], ident)
                            nc.vector.tensor_copy(
                                xT_t[:, kk, c * 128:(c + 1) * 128], xps)

                    for c in range(CHUNKS):
                        lhs = [xT_t[:, kk, c * 128:(c + 1) * 128] for kk in range(KK)]
                        # pieces 0-2: q|k (head-interleaved), pieces 3-4: v
                        for group in ((0, 1, 2), (3, 4)):
                            psums = {}
                            for p in group:
                                psums[p] = ps_mm.tile([128, 512], FP32, name="qkv_ps")
                            for kk in range(KK):
                                for p in group:
                                    width = 256 if p == 4 else 512
                                    if p < 3:
                                        rhs = w_qkT[:, kk, p * 512:p * 512 + width]
                                    else:
                                        off = (p - 3) * 512
                                        rhs = w_vT[:, kk, off:off + width]
                                    nc.tensor.matmul(
                                        psums[p][:, :width], lhs[kk], rhs,
                                        start=(kk == 0), stop=(kk == KK - 1))
                            for p in group:
                                width = 256 if p == 4 else 512
                                if p < 3:
                                    nc.vector.tensor_copy(
                                        qk_t[:, c, p * 512:p * 512 + width],
                                        psums[p][:, :width])
                                else:
                                    off = (p - 3) * 512
                                    nc.scalar.copy(
                                        v_t[:, c, off:off + width],
                                        psums[p][:, :width])

                    # covariance + Gram accumulation per head
                    for h in range(H):
                        # both matmuls share one psum bank: a single
                        # accumulation group (one start, one stop); start=True
                        # clears has_written for the whole bank, so the first
                        # write to each element overwrites.
                        cg_ps = ps_cg.tile([HD, 512], FP32, name="cg_ps")
                        for c in range(CHUNKS):
                            rhs = qk_t[:, c, 192 * h:192 * h + 256]
                            nc.tensor.matmul(
                                cg_ps[:, 0:256],
                                qk_t[:, c, 192 * h:192 * h + HD], rhs,
                                start=(c == 0), stop=False)
                            nc.tensor.matmul(
                                cg_ps[:, 256:512],
                                qk_t[:, c, 192 * h + HD:192 * h + 192], rhs,
                                start=False, stop=(c == CHUNKS - 1))
                        nc.vector.tensor_add(
                            cg_accum[:, h, :], cg_ps, cg_accum[:, h, :])

                    nc.sync.dma_start(
                        v_dram[t0:t0 + TOK_TILE, :].rearrange(
                            "(c p) f -> p c f", p=128),
                        v_t)

        # ---------------- phase 2: norms + softmax ----------------
        # cg_accum[:, h, :]: [0:96] Gq, [96:192] C, [352:448] Gk
        ident96 = ident[0:96, 0:96]
        sq = singles.tile([HD, 2, H], FP32)
        scr = singles.tile([HD, HD], FP32)
        for h in range(H):
            nc.vector.tensor_tensor(
                scr, cg_accum[:, h, 0:HD], ident96, mybir.AluOpType.mult)
            nc.vector.reduce_sum(sq[:, 0, h:h + 1], scr,
                                 axis=mybir.AxisListType.X)
            nc.vector.tensor_tensor(
                scr, cg_accum[:, h, 352:448], ident96, mybir.AluOpType.mult)
            nc.vector.reduce_sum(sq[:, 1, h:h + 1], scr,
                                 axis=mybir.AxisListType.X)

        nrm = singles.tile([HD, 2, H], FP32)
        nc.scalar.sqrt(nrm, sq)
        nc.vector.tensor_scalar_max(nrm, nrm, EPS)
        rnorm = singles.tile([HD, 2, H], FP32)
        nc.vector.reciprocal(rnorm, nrm)
        # fold temperature into the q-side scale
        rq = singles.tile([HD, H], FP32)
        nc.vector.tensor_tensor(rq, rnorm[:, 0, :], temp_all, mybir.AluOpType.mult)

        # rk transposed to the free dim (roundtrip through DRAM), broadcast
        # across partitions on the way back in
        rk_scr = dram.tile([HD, H], FP32)
        nc.sync.dma_start(rk_scr, rnorm[:, 1, :])
        rk_all = singles.tile([HD, H, HD], FP32)
        for h in range(H):
            nc.gpsimd.dma_start(
                rk_all[:, h, :],
                bass.AP(tensor=rk_scr.tensor, offset=rk_scr.offset + h,
                        ap=[[0, HD], [H, HD]]))

        for h in range(H):
            att = singles.tile([HD, HD], FP32, name="att", bufs=2)
            nc.vector.tensor_scalar_mul(
                att, cg_accum[:, h, HD:2 * HD], rq[:, h:h + 1])
            nc.vector.tensor_tensor(att, att, rk_all[:, h, :],
                                    mybir.AluOpType.mult)
            mx = singles.tile([HD, 1], FP32, name="mx", bufs=2)
            nc.vector.reduce_max(mx, att, axis=mybir.AxisListType.X)
            nmx = singles.tile([HD, 1], FP32, name="nmx", bufs=2)
            nc.vector.tensor_scalar_mul(nmx, mx, -1.0)
            se = singles.tile([HD, 1], FP32, name="se", bufs=2)
            nc.scalar.activation(att, att, mybir.ActivationFunctionType.Exp,
                                 bias=nmx, scale=1.0, accum_out=se)
            rse = singles.tile([HD, 1], FP32, name="rse", bufs=2)
            nc.vector.reciprocal(rse, se)
            nc.vector.tensor_scalar_mul(att, att, rse)
            atps = ps_tr.tile([HD, HD], FP32, name="atps", tag="tr")
            nc.tensor.transpose(atps, att, ident96)
            nc.vector.tensor_copy(attnT[:, h, :], atps)

        # ---------------- phase 3: attn@v + proj ----------------
        with tc.tile_pool(name="v2p", bufs=2) as v2p, \
             tc.tile_pool(name="vtp", bufs=2) as vtp, \
             tc.tile_pool(name="otp", bufs=1) as otp, \
             tc.tile_pool(name="yp", bufs=2) as yp, \
             tc.tile_pool(name="wpp", bufs=1) as wpp, \
             tc.tile_pool(name="wpload", bufs=2) as wpload, \
             tc.tile_pool(name="ps_vtr", bufs=2, space="PSUM") as ps_vtr, \
             tc.tile_pool(name="ps_o", bufs=2, space="PSUM") as ps_o, \
             tc.tile_pool(name="ps_y", bufs=2, space="PSUM") as ps_y:
            # w_proj (cout, c) -> w_projT (96 c-part per head, cout free)
            w_projT = wpp.tile([HD, H, C], FP32R)
            for n in range(C // 128):
                wp_blk = wpload.tile([128, C], FP32, name="wp_blk")
                nc.sync.dma_start(wp_blk, wproj_d[n * 128:(n + 1) * 128, :])
                for h in range(H):
                    tps2 = ps_tr.tile([HD, 128], FP32, name="tps2", tag="tr")
                    nc.tensor.transpose(
                        tps2, wp_blk[:, h * HD:(h + 1) * HD], ident)
                    nc.vector.tensor_copy(
                        w_projT[:, h, n * 128:(n + 1) * 128], tps2)

            for t in range(N_TILES):
                t0 = t * TOK_TILE
                v2_t = v2p.tile([128, CHUNKS, C], FP32, name="v2_t")
                nc.sync.dma_start(
                    v2_t,
                    v_dram[t0:t0 + TOK_TILE, :].rearrange("(c p) f -> p c f", p=128))
                vT_t = vtp.tile([HD, H, TOK_TILE], FP32R, name="vT_t")
                for h in range(H):
                    for c in range(CHUNKS):
                        vps = ps_vtr.tile([HD, 128], FP32, name="vps")
                        nc.tensor.transpose(
                            vps, v2_t[:, c, h * HD:(h + 1) * HD], ident)
                        nc.vector.tensor_copy(
                            vT_t[:, h, c * 128:(c + 1) * 128], vps)

                outT_t = otp.tile([HD, H, TOK_TILE], FP32R, name="outT_t")
                for h in range(H):
                    ops_ = ps_o.tile([HD, TOK_TILE], FP32, name="ops_")
                    nc.tensor.matmul(ops_, attnT[:, h, :], vT_t[:, h, :],
                                     start=True, stop=True)
                    nc.vector.tensor_copy(outT_t[:, h, :], ops_)

                y_t = yp.tile([128, CHUNKS, C], FP32, name="y_t")
                for c in range(CHUNKS):
                    for (off, width) in ((0, 512), (512, 256)):
                        yps = ps_y.tile([128, 512], FP32, name="yps")
                        for h in range(H):
                            nc.tensor.matmul(
                                yps[:, :width],
                                outT_t[:, h, c * 128:(c + 1) * 128],
                                w_projT[:, h, off:off + width],
                                start=(h == 0), stop=(h == H - 1))
                        nc.vector.tensor_tensor(
                            y_t[:, c, off:off + width], yps[:, :width],
                            b_all[:, off:off + width], mybir.AluOpType.add)
                nc.sync.dma_start(
                    out_d[t0:t0 + TOK_TILE, :].rearrange("(c p) f -> p c f", p=128),
                    y_t)


def _get_nc():
    global _CACHED_NC
    if _CACHED_NC is None:
        _CACHED_NC = build_nc()
    return _CACHED_NC


def kernel(x, w_qkv, temperature, w_proj, b_proj):
    nc = _get_nc()
    x = np.ascontiguousarray(np.asarray(x, dtype=np.float32))
    in_maps = []
    for b in range(8):
        in_maps.append({
            "x": x[b],
            "w_qkv": np.asarray(w_qkv, dtype=np.float32),
            "temperature": np.asarray(temperature, dtype=np.float32),
            "w_proj": np.asarray(w_proj, dtype=np.float32),
            "b_proj": np.asarray(b_proj, dtype=np.float32),
        })
    res = run_bass_kernel_spmd(nc, in_maps, core_ids=list(range(8)))
    return np.stack([r["out"] for r in res.results], axis=0)
